# revision 1
# baseline (speedup 1.0000x reference)
"""Trainium2 Bass kernel for CausalGraphLayer (GCN conv + causal attention mix).

out = D^{-1/2} (A+I) D^{-1/2} x @ (W @ softmax(CA, axis=1)) + b @ softmax(CA)

Strategy (8 NeuronCores, SPMD):
 - Shard destination nodes across cores (12500 each); partition edges by dst.
 - Replicate x and the small params to every core.
 - Host builds, per core, a slot table: dst nodes degree-sorted into blocks of
   128 (PSUM partitions); slot j of block b holds the j-th in-edge's source
   index and norm for each of the 128 dsts. Pad slots use an out-of-bounds
   index (descriptor skipped by HW) and norm=0.
 - Device: per slot column, one indirect DMA gathers x[src] rows ([128,1] ->
   [128,64], the HW-supported form); DVE multiplies by norms and seg-reduces
   over slots; PE applies M = W @ softmax(CA) and the bias row.
"""
import os
import numpy as np

NO_BC = bool(os.environ.get("KERNEL_NB"))

import concourse.bass as bass
import concourse.bacc as bacc
import concourse.mybir as mybir
import concourse.tile as tile
from concourse.bass_utils import run_bass_kernel_spmd

P = 128
D = 64
N_CORES = 8
OOB_IDX = 1 << 20

LAST_EXEC_NS = None


def _build_nc(N, n_blocks, s_list, col_off, ST):
    nc = bacc.Bacc(None, target_bir_lowering=False)
    f32 = mybir.dt.float32
    x = nc.declare_dram_parameter("x", [N, D], f32, isOutput=False)
    offs = nc.declare_dram_parameter("offs", [P, ST], mybir.dt.int32, isOutput=False)
    norms = nc.declare_dram_parameter("norms", [P, ST], f32, isOutput=False)
    wmat = nc.declare_dram_parameter("wmat", [D, D], f32, isOutput=False)
    bvec = nc.declare_dram_parameter("bvec", [D, 1], f32, isOutput=False)
    cattn = nc.declare_dram_parameter("cattn", [D, D], f32, isOutput=False)
    ident = nc.declare_dram_parameter("ident", [P, P], f32, isOutput=False)
    out = nc.declare_dram_parameter("out", [n_blocks * P, D], f32, isOutput=True)

    s_max = max(s_list)

    with tile.TileContext(nc) as tc:
        with (
            tc.tile_pool(name="const", bufs=1) as cpool,
            tc.tile_pool(name="psum", bufs=2, space="PSUM") as ppool,
            tc.tile_pool(name="work", bufs=3) as wpool,
            tc.tile_pool(name="outp", bufs=3) as opool,
        ):
            offs_s = cpool.tile([P, ST], mybir.dt.int32)
            norms_s = cpool.tile([P, ST], f32)
            nc.sync.dma_start(out=offs_s[:], in_=offs[:, :])
            nc.sync.dma_start(out=norms_s[:], in_=norms[:, :])
            id_s = cpool.tile([P, P], f32)
            nc.sync.dma_start(out=id_s[:], in_=ident[:, :])
            w_s = cpool.tile([D, D], f32)
            nc.sync.dma_start(out=w_s[:], in_=wmat[:, :])
            b_s = cpool.tile([D, 1], f32)
            nc.sync.dma_start(out=b_s[:], in_=bvec[:, :])
            ca_s = cpool.tile([D, D], f32)
            nc.sync.dma_start(out=ca_s[:], in_=cattn[:, :])

            # ---- softmax(CA, axis=1) in-place on ca_s ----
            mx = cpool.tile([D, 1], f32)
            nc.vector.tensor_reduce(out=mx[:], in_=ca_s[:], axis=mybir.AxisListType.X,
                                    op=mybir.AluOpType.max)
            nc.vector.tensor_scalar_mul(mx[:], mx[:], -1.0)
            nc.scalar.activation(out=ca_s[:], in_=ca_s[:],
                                 func=mybir.ActivationFunctionType.Exp,
                                 bias=mx[:, :1], scale=1.0)
            sm = cpool.tile([D, 1], f32)
            nc.vector.tensor_reduce(out=sm[:], in_=ca_s[:], axis=mybir.AxisListType.X,
                                    op=mybir.AluOpType.add)
            rc = cpool.tile([D, 1], f32)
            nc.vector.reciprocal(rc[:], sm[:])
            nc.vector.tensor_scalar_mul(ca_s[:], ca_s[:], rc[:, :1])

            # ---- M = W @ softmax(CA);  bS = b.T @ softmax(CA) ----
            wt_p = ppool.tile([D, D], f32, tag="pa")
            nc.tensor.transpose(wt_p[:], w_s[:], id_s[:D, :D])
            wt_s = cpool.tile([D, D], f32)
            nc.vector.tensor_copy(out=wt_s[:], in_=wt_p[:])
            m_p = ppool.tile([D, D], f32, tag="pa")
            nc.tensor.matmul(m_p[:], wt_s[:], ca_s[:], start=True, stop=True)
            m_s = cpool.tile([D, D], f32)
            nc.vector.tensor_copy(out=m_s[:], in_=m_p[:])
            bs_p = ppool.tile([1, D], f32, tag="pa")
            nc.tensor.matmul(bs_p[:], b_s[:, :1], ca_s[:], start=True, stop=True)
            bs_s = cpool.tile([1, D], f32)
            nc.vector.tensor_copy(out=bs_s[:], in_=bs_p[:])
            ones_s = cpool.tile([1, P], f32)
            nc.vector.memset(ones_s[:], 1.0)

            # ---- main loop over dst blocks ----
            for b in range(n_blocks):
                S = s_list[b]
                c0 = col_off[b]
                feat = wpool.tile([P, s_max * D], f32, tag="feat")
                if b < 3:
                    nc.vector.memset(feat[:], 0.0)
                for j in range(S):
                    nc.gpsimd.indirect_dma_start(
                        out=feat[:, j * D:(j + 1) * D],
                        out_offset=None,
                        in_=x[:, :],
                        in_offset=bass.IndirectOffsetOnAxis(
                            ap=offs_s[:, c0 + j:c0 + j + 1], axis=0),
                        bounds_check=None if NO_BC else N - 1,
                        oob_is_err=False,
                    )
                feat3 = feat[:, :S * D].rearrange("p (s d) -> p s d", s=S)
                nb = norms_s[:, c0:c0 + S].unsqueeze(2).to_broadcast([P, S, D])
                nc.vector.tensor_tensor(out=feat3, in0=feat3, in1=nb,
                                        op=mybir.AluOpType.mult)
                agg = opool.tile([P, D], f32, tag="agg")
                nc.vector.tensor_reduce(
                    out=agg[:], in_=feat[:, :S * D].rearrange("p (s d) -> p d s", s=S),
                    axis=mybir.AxisListType.X, op=mybir.AluOpType.add)
                # out_block = agg @ M + 1s*bS  (via aggT)
                t_p = ppool.tile([D, P], f32, tag="pt")
                nc.tensor.transpose(t_p[:], agg[:], id_s[:, :])
                aggT = opool.tile([D, P], f32, tag="aggT")
                nc.vector.tensor_copy(out=aggT[:], in_=t_p[:])
                o_p = ppool.tile([P, D], f32, tag="po")
                nc.tensor.matmul(o_p[:], aggT[:], m_s[:], start=True, stop=False)
                nc.tensor.matmul(o_p[:], ones_s[:, :], bs_s[:, :], start=False,
                                 stop=True, skip_group_check=True)
                o_s = opool.tile([P, D], f32, tag="os")
                nc.vector.tensor_copy(out=o_s[:], in_=o_p[:])
                nc.sync.dma_start(out=out[b * P:(b + 1) * P, :], in_=o_s[:])
    nc.compile()
    return nc


def kernel(x, edge_index, W, b, causal_attention, L=1, **_unused):
    global LAST_EXEC_NS
    x = np.ascontiguousarray(np.asarray(x, dtype=np.float32))
    ei = np.asarray(edge_index, dtype=np.int64)
    W = np.asarray(W, dtype=np.float32)
    bb = np.asarray(b, dtype=np.float32).reshape(D, 1)
    ca = np.asarray(causal_attention, dtype=np.float32)
    N = x.shape[0]
    src, dst = ei[0].astype(np.int64), ei[1].astype(np.int64)

    # GCN normalization (index-only math)
    deg = np.bincount(dst, minlength=N).astype(np.float64) + 1.0
    dinv = (1.0 / np.sqrt(deg)).astype(np.float32)
    norm_e = dinv[src] * dinv[dst]

    n_per = N // N_CORES
    n_blocks = (n_per + P - 1) // P

    # per-core degree-sorted dst ordering and slot tables
    cores = []
    for c in range(N_CORES):
        lo, hi = c * n_per, (c + 1) * n_per
        sel = (dst >= lo) & (dst < hi)
        s_c, d_c, w_c = src[sel], dst[sel] - lo, norm_e[sel]
        degc = np.bincount(d_c, minlength=n_per) + 1  # incl self loop
        order = np.argsort(-degc, kind="stable")      # dst local ids, degree desc
        rank = np.empty(n_per, np.int64)
        rank[order] = np.arange(n_per)
        cores.append((lo, s_c, d_c, w_c, degc, order, rank))

    # uniform per-block slot counts across cores
    s_list = []
    for bidx in range(n_blocks):
        m = 1
        for (_, _, _, _, degc, order, _) in cores:
            i0 = bidx * P
            if i0 < n_per:
                m = max(m, int(degc[order[i0]]))
        s_list.append(m)
    col_off = np.concatenate([[0], np.cumsum(s_list)]).astype(np.int64)
    ST = int(col_off[-1])

    in_maps = []
    perms = []
    for c in range(N_CORES):
        lo, s_c, d_c, w_c, degc, order, rank = cores[c]
        offs_arr = np.full((P, ST), 0 if NO_BC else OOB_IDX, dtype=np.int32)
        norms_arr = np.zeros((P, ST), dtype=np.float32)

        # self loops: slot 0 of every dst
        r_all = rank  # rank of local dst i
        p_all = (r_all % P).astype(np.int64)
        blk_all = r_all // P
        cols0 = col_off[blk_all]
        offs_arr[p_all, cols0] = (np.arange(n_per) + lo).astype(np.int32)
        norms_arr[p_all, cols0] = dinv[lo:lo + n_per] ** 2

        # edges: slots 1.. per dst in rank order
        rk = rank[d_c]
        o2 = np.argsort(rk, kind="stable")
        rk_s, s_s, w_s_ = rk[o2], s_c[o2], w_c[o2]
        # position within group
        grp_start = np.searchsorted(rk_s, np.arange(n_per), side="left")
        j_in = np.arange(len(rk_s)) - grp_start[rk_s]
        cols = col_off[rk_s // P] + 1 + j_in
        rows = rk_s % P
        offs_arr[rows, cols] = s_s.astype(np.int32)
        norms_arr[rows, cols] = w_s_

        in_maps.append({
            "x": x, "offs": offs_arr, "norms": norms_arr,
            "wmat": W, "bvec": bb, "cattn": ca,
            "ident": np.eye(P, dtype=np.float32),
        })
        perms.append(order + lo)

    nc = _build_nc(N, n_blocks, s_list, col_off, ST)

    trace = bool(os.environ.get("KERNEL_TRACE"))
    if trace:
        try:
            import ntff_shim  # noqa: F401
        except Exception:
            trace = False
    r = run_bass_kernel_spmd(nc, in_maps, list(range(N_CORES)), trace=trace)
    LAST_EXEC_NS = r.exec_time_ns

    out = np.empty((N, D), dtype=np.float32)
    for c in range(N_CORES):
        out[perms[c]] = r.results[c]["out"][:n_per]
    return out



# revision 6
# speedup vs baseline: 1.6660x; 1.6660x over previous
"""Trainium2 Bass kernel for CausalGraphLayer (GCN conv + causal attention mix).

out = D^{-1/2} (A+I) D^{-1/2} x @ (W @ softmax(CA, axis=1)) + b @ softmax(CA)

Strategy (8 NeuronCores, SPMD):
 - Shard destination nodes across cores; partition edges by dst. Replicate x
   and the small params to every core.
 - Host packs dsts into blocks of 128 (greedy 4-dim vector bin packing over
   per-bank in-degrees), snake-deals blocks to cores so the per-position slot
   counts are nearly identical across cores, then uniformizes (max) so one
   SPMD program fits all cores.
 - The gather uses the microcoded SWDGE dma_gather (InstDMAGatherAnt,
   ~0.34ns/row descriptor gen) instead of per-slot indirect DMAs (~10ns/row
   software DGE) - x is split into 4 banks of 25000 rows because gather
   indices are int16.
 - Per (block, bank): grid [128 dst-lanes, S_k(b) slots, 64] gathered rows;
   DVE multiplies by per-edge norms (padding cells have norm 0, idx 0) and
   seg-reduces slots; 4 bank partials reduce again; PE applies
   M = W @ softmax(CA) and the bias row.
"""
import os
import numpy as np

import concourse.bass as bass
import concourse.bacc as bacc
import concourse.mybir as mybir
import concourse.tile as tile
from concourse.bass_utils import run_bass_kernel_spmd
from concourse.library_config import mlp

P = 128
D = 64
N_CORES = 8
NBANK = 4
BANK_ROWS = 25000

LAST_EXEC_NS = None


def _build_nc(N, uni, chunks):
    """uni: [n_blocks, NBANK] slot counts (shared across cores).
    chunks: list of (b0, b1) block ranges."""
    n_blocks = uni.shape[0]
    ck_off = np.concatenate([[0], np.cumsum(uni.sum(0))])  # not used; per-bank below
    col_off = np.zeros((n_blocks + 1, NBANK), np.int64)
    col_off[1:] = np.cumsum(uni, axis=0)
    ST = col_off[-1]  # per-bank total cols
    nc = bacc.Bacc(None, target_bir_lowering=False, num_swdge_queues=4)
    f32 = mybir.dt.float32
    i16 = mybir.dt.int16
    x = nc.declare_dram_parameter("x", [N, D], f32, isOutput=False)
    idxs = [
        nc.declare_dram_parameter(f"idx{k}", [P, int(ST[k]) * 8], i16, isOutput=False)
        for k in range(NBANK)
    ]
    norms = [
        nc.declare_dram_parameter(f"norms{k}", [P, int(ST[k])], f32, isOutput=False)
        for k in range(NBANK)
    ]
    wmat = nc.declare_dram_parameter("wmat", [D, D], f32, isOutput=False)
    bvec = nc.declare_dram_parameter("bvec", [D, 1], f32, isOutput=False)
    cattn = nc.declare_dram_parameter("cattn", [D, D], f32, isOutput=False)
    ident = nc.declare_dram_parameter("ident", [P, P], f32, isOutput=False)
    out = nc.declare_dram_parameter("out", [n_blocks * P, D], f32, isOutput=True)

    max_chunk_cols = [
        max(int(col_off[b1, k] - col_off[b0, k]) for (b0, b1) in chunks)
        for k in range(NBANK)
    ]

    with tile.TileContext(nc) as tc:
        with (
            tc.tile_pool(name="const", bufs=1) as cpool,
            tc.tile_pool(name="psum", bufs=2, space="PSUM") as ppool,
            tc.tile_pool(name="grid", bufs=2) as gpool,
            tc.tile_pool(name="outp", bufs=3) as opool,
        ):
            nc.gpsimd.load_library(mlp)
            idx_s = []
            norm_s = []
            for k in range(NBANK):
                t = cpool.tile([P, int(ST[k]) * 8], i16, tag=f"idx{k}")
                nc.sync.dma_start(out=t[:], in_=idxs[k][:, :])
                idx_s.append(t)
                t = cpool.tile([P, int(ST[k])], f32, tag=f"norms{k}")
                nc.sync.dma_start(out=t[:], in_=norms[k][:, :])
                norm_s.append(t)
            id_s = cpool.tile([P, P], f32)
            nc.sync.dma_start(out=id_s[:], in_=ident[:, :])
            w_s = cpool.tile([D, D], f32)
            nc.sync.dma_start(out=w_s[:], in_=wmat[:, :])
            b_s = cpool.tile([D, 1], f32)
            nc.sync.dma_start(out=b_s[:], in_=bvec[:, :])
            ca_s = cpool.tile([D, D], f32)
            nc.sync.dma_start(out=ca_s[:], in_=cattn[:, :])

            # ---- softmax(CA, axis=1) in-place on ca_s ----
            mx = cpool.tile([D, 1], f32)
            nc.vector.tensor_reduce(out=mx[:], in_=ca_s[:], axis=mybir.AxisListType.X,
                                    op=mybir.AluOpType.max)
            nc.vector.tensor_scalar_mul(mx[:], mx[:], -1.0)
            nc.scalar.activation(out=ca_s[:], in_=ca_s[:],
                                 func=mybir.ActivationFunctionType.Exp,
                                 bias=mx[:, :1], scale=1.0)
            sm = cpool.tile([D, 1], f32)
            nc.vector.tensor_reduce(out=sm[:], in_=ca_s[:], axis=mybir.AxisListType.X,
                                    op=mybir.AluOpType.add)
            rc = cpool.tile([D, 1], f32)
            nc.vector.reciprocal(rc[:], sm[:])
            nc.vector.tensor_scalar_mul(ca_s[:], ca_s[:], rc[:, :1])

            # ---- M = W @ softmax(CA);  bS = b.T @ softmax(CA) ----
            wt_p = ppool.tile([D, D], f32, tag="pa")
            nc.tensor.transpose(wt_p[:], w_s[:], id_s[:D, :D])
            wt_s = cpool.tile([D, D], f32)
            nc.vector.tensor_copy(out=wt_s[:], in_=wt_p[:])
            m_p = ppool.tile([D, D], f32, tag="pa")
            nc.tensor.matmul(m_p[:], wt_s[:], ca_s[:], start=True, stop=True)
            m_s = cpool.tile([D, D], f32)
            nc.vector.tensor_copy(out=m_s[:], in_=m_p[:])
            bs_p = ppool.tile([1, D], f32, tag="pa")
            nc.tensor.matmul(bs_p[:], b_s[:, :1], ca_s[:], start=True, stop=True)
            bs_s = cpool.tile([1, D], f32)
            nc.vector.tensor_copy(out=bs_s[:], in_=bs_p[:])
            ones_s = cpool.tile([1, P], f32)
            nc.vector.memset(ones_s[:], 1.0)

            # ---- main loop over chunks of dst blocks ----
            for (b0, b1) in chunks:
                grids = []
                for k in range(NBANK):
                    c0, c1 = int(col_off[b0, k]), int(col_off[b1, k])
                    cols = c1 - c0
                    g = gpool.tile([P, max_chunk_cols[k] * D], f32, tag=f"g{k}")
                    if cols > 0:
                        # ucode SWDGE ring caps one gather at 1024 rows (8 cols)
                        for cc0 in range(0, cols, 8):
                            cc1 = min(cc0 + 8, cols)
                            num = (cc1 - cc0) * P
                            nc.gpsimd.dma_gather(
                                g[:, cc0 * D:cc1 * D].rearrange(
                                    "p (c d) -> p c d", d=D),
                                x[k * BANK_ROWS:(k + 1) * BANK_ROWS, :],
                                idx_s[k][:, (c0 + cc0) * 8:(c0 + cc1) * 8],
                                num,
                                num,
                                D,
                                queue_num=k,
                            )
                        # multiply by norms (padding slots have norm 0)
                        nb = norm_s[k][:, c0:c1].unsqueeze(2).to_broadcast(
                            [P, cols, D])
                        g3 = g[:, :cols * D].rearrange("p (c d) -> p c d", d=D)
                        nc.vector.tensor_tensor(out=g3, in0=g3, in1=nb,
                                                op=mybir.AluOpType.mult)
                    grids.append(g)
                for b in range(b0, b1):
                    # per-bank partial reduction into tmp [P, NBANK*D]
                    tmp = opool.tile([P, NBANK * D], f32, tag="tmp")
                    for k in range(NBANK):
                        S = int(uni[b, k])
                        o0 = int(col_off[b, k] - col_off[b0, k])
                        if S == 0:
                            nc.vector.memset(tmp[:, k * D:(k + 1) * D], 0.0)
                            continue
                        nc.vector.tensor_reduce(
                            out=tmp[:, k * D:(k + 1) * D],
                            in_=grids[k][:, o0 * D:(o0 + S) * D].rearrange(
                                "p (s d) -> p d s", s=S),
                            axis=mybir.AxisListType.X, op=mybir.AluOpType.add)
                    agg = opool.tile([P, D], f32, tag="agg")
                    nc.vector.tensor_reduce(
                        out=agg[:],
                        in_=tmp[:].rearrange("p (k d) -> p d k", k=NBANK),
                        axis=mybir.AxisListType.X, op=mybir.AluOpType.add)
                    # out_block = agg @ M + 1s*bS  (via aggT)
                    t_p = ppool.tile([D, P], f32, tag="pt")
                    nc.tensor.transpose(t_p[:], agg[:], id_s[:, :])
                    aggT = opool.tile([D, P], f32, tag="aggT")
                    nc.vector.tensor_copy(out=aggT[:], in_=t_p[:])
                    o_p = ppool.tile([P, D], f32, tag="po")
                    nc.tensor.matmul(o_p[:], aggT[:], m_s[:], start=True, stop=False)
                    nc.tensor.matmul(o_p[:], ones_s[:, :], bs_s[:, :], start=False,
                                     stop=True, skip_group_check=True)
                    o_s = opool.tile([P, D], f32, tag="os")
                    nc.vector.tensor_copy(out=o_s[:], in_=o_p[:])
                    nc.sync.dma_start(out=out[b * P:(b + 1) * P, :], in_=o_s[:])
    nc.compile()
    return nc


def _greedy_pack(dd, nb):
    """Greedy 4-dim vector packing of len(dd) items into nb bins of <=128."""
    n = dd.shape[0]
    order = np.argsort(
        -(dd.max(1).astype(np.int64) * 1000 + dd.sum(1)), kind="stable")
    caps = np.zeros((nb, NBANK), np.int32)
    cnt = np.zeros(nb, np.int32)
    capsum = np.zeros(nb, np.int64)
    assign = np.empty(n, np.int32)
    big = 1 << 30
    for i in order:
        v = dd[i]
        inc = np.maximum(caps, v).sum(1) - capsum
        inc[cnt >= P] = big
        j = int(np.argmin(inc))
        caps[j] = np.maximum(caps[j], v)
        capsum[j] = caps[j].sum()
        cnt[j] += 1
        assign[i] = j
    return caps, assign


def kernel(x, edge_index, W, b, causal_attention, L=1, **_unused):
    global LAST_EXEC_NS
    x = np.ascontiguousarray(np.asarray(x, dtype=np.float32))
    ei = np.asarray(edge_index, dtype=np.int64)
    W = np.asarray(W, dtype=np.float32)
    bb = np.asarray(b, dtype=np.float32).reshape(D, 1)
    ca = np.asarray(causal_attention, dtype=np.float32)
    N = x.shape[0]
    src, dst = ei[0], ei[1]

    # GCN normalization (index-only math)
    deg = np.bincount(dst, minlength=N).astype(np.float64) + 1.0
    dinv = (1.0 / np.sqrt(deg)).astype(np.float32)

    # per-dst per-bank in-degree (self loop counts in bank(dst))
    bank_e = (src // BANK_ROWS).astype(np.int32)
    degkb = np.zeros((N, NBANK), np.int32)
    for k in range(NBANK):
        np.add.at(degkb[:, k], dst[bank_e == k], 1)
    degkb[np.arange(N), np.arange(N) // BANK_ROWS] += 1

    n_per = N // N_CORES
    n_blocks = (n_per + P - 1) // P
    total_blocks = n_blocks * N_CORES

    # greedy pack in 8 slices (speed), then snake-deal blocks to cores
    all_caps = np.zeros((total_blocks, NBANK), np.int32)
    assign = np.empty(N, np.int64)  # dst -> global block id
    for g in range(N_CORES):
        sl = slice(g * n_per, (g + 1) * n_per)
        caps, asg = _greedy_pack(degkb[sl], n_blocks)
        all_caps[g * n_blocks:(g + 1) * n_blocks] = caps
        assign[sl] = asg + g * n_blocks
    o = np.argsort(-all_caps.sum(1), kind="stable")
    # dealt[c, r] = global block id at position r on core c (snake)
    dealt = np.empty((N_CORES, n_blocks), np.int64)
    for r in range(n_blocks):
        blocks = o[r * N_CORES:(r + 1) * N_CORES]
        if r % 2 == 1:
            blocks = blocks[::-1]
        dealt[:, r] = blocks
    uni = all_caps[dealt].max(axis=0)  # [n_blocks, NBANK]
    col_off = np.zeros((n_blocks + 1, NBANK), np.int64)
    col_off[1:] = np.cumsum(uni, axis=0)
    ST = col_off[-1]

    # map: global block id -> (core, position)
    blk_core = np.empty(total_blocks, np.int64)
    blk_pos = np.empty(total_blocks, np.int64)
    for c in range(N_CORES):
        blk_core[dealt[c]] = c
        blk_pos[dealt[c]] = np.arange(n_blocks)

    # lane assignment: dsts of each global block get lanes 0..cnt-1
    # order dsts by (core, pos, dst id)
    dst_block = assign  # global block per dst
    keys = blk_core[dst_block] * (n_blocks * P * 8) + blk_pos[dst_block] * P * 8
    order_d = np.argsort(keys, kind="stable")  # dsts grouped by (core, pos)
    # lane = index within group
    gb_sorted = dst_block[order_d]
    grp_start_idx = np.searchsorted(
        blk_core[gb_sorted] * n_blocks + blk_pos[gb_sorted],
        np.arange(total_blocks), side="left")
    lane_sorted = np.arange(N) - grp_start_idx[
        blk_core[gb_sorted] * n_blocks + blk_pos[gb_sorted]]
    lane = np.empty(N, np.int64)
    lane[order_d] = lane_sorted
    core_of = blk_core[dst_block]
    pos_of = blk_pos[dst_block]

    # per-edge norm values; self loops appended as edges
    norm_e = dinv[src] * dinv[dst]
    loops = np.arange(N)
    src_all = np.concatenate([loops, src])   # self loops FIRST (slot 0)
    dst_all = np.concatenate([loops, dst])
    w_all = np.concatenate([(dinv * dinv).astype(np.float32),
                            norm_e.astype(np.float32)])
    bank_all = (src_all // BANK_ROWS).astype(np.int32)

    # chunk blocks by total column budget (SBUF: grid tiles are
    # budget*256B per partition per buffer, double buffered)
    COL_BUDGET = 160
    chunks = []
    b0 = 0
    while b0 < n_blocks:
        b1 = b0 + 1
        while b1 < n_blocks and uni[b0:b1 + 1].sum() <= COL_BUDGET:
            b1 += 1
        chunks.append((b0, b1))
        b0 = b1

    nc = _build_nc(N, uni, chunks)

    in_maps = []
    perms = []  # per core: row index in out -> global dst id
    ecore = core_of[dst_all]
    epos = pos_of[dst_all]
    elane = lane[dst_all]
    for c in range(N_CORES):
        im = {"x": x, "wmat": W, "bvec": bb, "cattn": ca,
              "ident": np.eye(P, dtype=np.float32)}
        esel = ecore == c
        for k in range(NBANK):
            sel = esel & (bank_all == k)
            s_k = src_all[sel] - k * BANK_ROWS
            w_k = w_all[sel]
            pos_k = epos[sel]
            lane_k = elane[sel]
            # sort by (pos, lane) stable to get slot j within (pos, lane)
            rank = pos_k * P + lane_k
            o2 = np.argsort(rank, kind="stable")
            rank_s, s_s, w_s_, pos_s, lane_s = (
                rank[o2], s_k[o2], w_k[o2], pos_k[o2], lane_k[o2])
            grp = np.searchsorted(rank_s, np.arange(n_blocks * P), side="left")
            j_in = np.arange(len(rank_s)) - grp[rank_s]
            cols = col_off[pos_s, k] + j_in
            st_k = int(ST[k])
            offs_lin = np.zeros(st_k * P, np.int16)
            norms_arr = np.zeros((P, st_k), np.float32)
            offs_lin[cols * P + lane_s] = s_s.astype(np.int16)
            norms_arr[lane_s, cols] = w_s_
            # wrap idx: linear i -> [i%16 (+16r), i//16]
            wrapped = offs_lin.reshape(st_k * P // 16, 16).T  # [16, numk/16]
            idx_arr = np.tile(wrapped, (8, 1))
            im[f"idx{k}"] = np.ascontiguousarray(idx_arr)
            im[f"norms{k}"] = norms_arr
        in_maps.append(im)
        # perm: (pos, lane) -> dst global id
        mine = core_of == c
        pm = np.full(n_blocks * P, -1, np.int64)
        pm[pos_of[mine] * P + lane[mine]] = np.nonzero(mine)[0]
        perms.append(pm)

    trace = bool(os.environ.get("KERNEL_TRACE"))
    if trace:
        try:
            import ntff_shim  # noqa: F401
        except Exception:
            trace = False
    r = run_bass_kernel_spmd(nc, in_maps, list(range(N_CORES)), trace=trace)
    LAST_EXEC_NS = r.exec_time_ns

    out = np.empty((N, D), dtype=np.float32)
    for c in range(N_CORES):
        pm = perms[c]
        valid = pm >= 0
        out[pm[valid]] = r.results[c]["out"][valid]
    return out


# revision 8
# speedup vs baseline: 2.2541x; 1.3530x over previous
"""Trainium2 Bass kernel for CausalGraphLayer (GCN conv + causal attention mix).

out = D^{-1/2} (A+I) D^{-1/2} x @ (W @ softmax(CA, axis=1)) + b @ softmax(CA)

Strategy (8 NeuronCores, SPMD):
 - Shard destination nodes across cores; partition edges by dst. Replicate x
   and the small params to every core.
 - Host packs dsts into blocks of 128 (greedy 4-dim vector bin packing over
   per-bank in-degrees), snake-deals blocks to cores so the per-position slot
   counts are nearly identical across cores, then uniformizes (max) so one
   SPMD program fits all cores.
 - The gather uses the microcoded SWDGE dma_gather (InstDMAGatherAnt,
   ~0.34ns/row descriptor gen) instead of per-slot indirect DMAs (~10ns/row
   software DGE) - x is split into 4 banks of 25000 rows because gather
   indices are int16.
 - Per (block, bank): grid [128 dst-lanes, S_k(b) slots, 64] gathered rows;
   DVE multiplies by per-edge norms (padding cells have norm 0, idx 0) and
   seg-reduces slots; 4 bank partials reduce again; PE applies
   M = W @ softmax(CA) and the bias row.
"""
import os
import numpy as np

import concourse.bass as bass
import concourse.bacc as bacc
import concourse.mybir as mybir
import concourse.tile as tile
from concourse.bass_utils import run_bass_kernel_spmd
from concourse.library_config import mlp

P = 128
D = 64
N_CORES = 8
NBANK = 4
BANK_ROWS = 25000

LAST_EXEC_NS = None


def _build_nc(N, uni, chunks):
    """uni: [n_blocks, NBANK] slot counts (shared across cores).
    chunks: list of (b0, b1) block ranges."""
    n_blocks = uni.shape[0]
    ck_off = np.concatenate([[0], np.cumsum(uni.sum(0))])  # not used; per-bank below
    col_off = np.zeros((n_blocks + 1, NBANK), np.int64)
    col_off[1:] = np.cumsum(uni, axis=0)
    ST = col_off[-1]  # per-bank total cols
    nc = bacc.Bacc(None, target_bir_lowering=False, num_swdge_queues=4)
    f32 = mybir.dt.float32
    i16 = mybir.dt.int16
    x = nc.declare_dram_parameter("x", [N, D], f32, isOutput=False)
    idxs = [
        nc.declare_dram_parameter(f"idx{k}", [P, int(ST[k]) * 8], i16, isOutput=False)
        for k in range(NBANK)
    ]
    norms = [
        nc.declare_dram_parameter(f"norms{k}", [P, int(ST[k])], f32, isOutput=False)
        for k in range(NBANK)
    ]
    wmat = nc.declare_dram_parameter("wmat", [D, D], f32, isOutput=False)
    bvec = nc.declare_dram_parameter("bvec", [D, 1], f32, isOutput=False)
    cattn = nc.declare_dram_parameter("cattn", [D, D], f32, isOutput=False)
    ident = nc.declare_dram_parameter("ident", [P, P], f32, isOutput=False)
    out = nc.declare_dram_parameter("out", [n_blocks * P, D], f32, isOutput=True)

    max_chunk_cols = [
        max(int(col_off[b1, k] - col_off[b0, k]) for (b0, b1) in chunks)
        for k in range(NBANK)
    ]

    with tile.TileContext(nc) as tc:
        with (
            tc.tile_pool(name="const", bufs=1) as cpool,
            tc.tile_pool(name="psum", bufs=2, space="PSUM") as ppool,
            tc.tile_pool(name="grid", bufs=2) as gpool,
            tc.tile_pool(name="outp", bufs=3) as opool,
        ):
            nc.gpsimd.load_library(mlp)
            idx_s = []
            norm_s = []
            for k in range(NBANK):
                t = cpool.tile([P, int(ST[k]) * 8], i16, tag=f"idx{k}")
                nc.sync.dma_start(out=t[:], in_=idxs[k][:, :])
                idx_s.append(t)
                t = cpool.tile([P, int(ST[k])], f32, tag=f"norms{k}")
                nc.sync.dma_start(out=t[:], in_=norms[k][:, :])
                norm_s.append(t)
            id_s = cpool.tile([P, P], f32)
            nc.sync.dma_start(out=id_s[:], in_=ident[:, :])
            w_s = cpool.tile([D, D], f32)
            nc.sync.dma_start(out=w_s[:], in_=wmat[:, :])
            b_s = cpool.tile([D, 1], f32)
            nc.sync.dma_start(out=b_s[:], in_=bvec[:, :])
            ca_s = cpool.tile([D, D], f32)
            nc.sync.dma_start(out=ca_s[:], in_=cattn[:, :])

            # ---- softmax(CA, axis=1) in-place on ca_s ----
            mx = cpool.tile([D, 1], f32)
            nc.vector.tensor_reduce(out=mx[:], in_=ca_s[:], axis=mybir.AxisListType.X,
                                    op=mybir.AluOpType.max)
            nc.vector.tensor_scalar_mul(mx[:], mx[:], -1.0)
            nc.scalar.activation(out=ca_s[:], in_=ca_s[:],
                                 func=mybir.ActivationFunctionType.Exp,
                                 bias=mx[:, :1], scale=1.0)
            sm = cpool.tile([D, 1], f32)
            nc.vector.tensor_reduce(out=sm[:], in_=ca_s[:], axis=mybir.AxisListType.X,
                                    op=mybir.AluOpType.add)
            rc = cpool.tile([D, 1], f32)
            nc.vector.reciprocal(rc[:], sm[:])
            nc.vector.tensor_scalar_mul(ca_s[:], ca_s[:], rc[:, :1])

            # ---- M = W @ softmax(CA);  bS = b.T @ softmax(CA) ----
            wt_p = ppool.tile([D, D], f32, tag="pa")
            nc.tensor.transpose(wt_p[:], w_s[:], id_s[:D, :D])
            wt_s = cpool.tile([D, D], f32)
            nc.vector.tensor_copy(out=wt_s[:], in_=wt_p[:])
            m_p = ppool.tile([D, D], f32, tag="pa")
            nc.tensor.matmul(m_p[:], wt_s[:], ca_s[:], start=True, stop=True)
            m_s = cpool.tile([D, D], f32)
            nc.vector.tensor_copy(out=m_s[:], in_=m_p[:])
            bs_p = ppool.tile([1, D], f32, tag="pa")
            nc.tensor.matmul(bs_p[:], b_s[:, :1], ca_s[:], start=True, stop=True)
            bs_s = cpool.tile([1, D], f32)
            nc.vector.tensor_copy(out=bs_s[:], in_=bs_p[:])
            ones_s = cpool.tile([1, P], f32)
            nc.vector.memset(ones_s[:], 1.0)

            # ---- main loop over chunks of dst blocks ----
            for (b0, b1) in chunks:
                grids = [
                    gpool.tile([P, max_chunk_cols[k] * D], f32, tag=f"g{k}",
                               name=f"grid{k}")
                    for k in range(NBANK)
                ]
                # round-robin gather calls across banks so the 4 SWDGE
                # queues' DMAs overlap (each queue's ring fits one
                # 1024-row call; same-queue calls serialize)
                calls = []
                for k in range(NBANK):
                    c0, c1 = int(col_off[b0, k]), int(col_off[b1, k])
                    sub = []
                    for cc0 in range(0, c1 - c0, 8):
                        sub.append((k, c0, cc0, min(cc0 + 8, c1 - c0)))
                    calls.append(sub)
                for i in range(max(len(s) for s in calls)):
                    for k in range(NBANK):
                        if i >= len(calls[k]):
                            continue
                        _, c0, cc0, cc1 = calls[k][i]
                        num = (cc1 - cc0) * P
                        nc.gpsimd.dma_gather(
                            grids[k][:, cc0 * D:cc1 * D].rearrange(
                                "p (c d) -> p c d", d=D),
                            x[k * BANK_ROWS:(k + 1) * BANK_ROWS, :],
                            idx_s[k][:, (c0 + cc0) * 8:(c0 + cc1) * 8],
                            num,
                            num,
                            D,
                            queue_num=k,
                        )
                for k in range(NBANK):
                    c0, c1 = int(col_off[b0, k]), int(col_off[b1, k])
                    cols = c1 - c0
                    if cols > 0:
                        # multiply by norms (padding slots have norm 0)
                        nb = norm_s[k][:, c0:c1].unsqueeze(2).to_broadcast(
                            [P, cols, D])
                        g3 = grids[k][:, :cols * D].rearrange(
                            "p (c d) -> p c d", d=D)
                        nc.vector.tensor_tensor(out=g3, in0=g3, in1=nb,
                                                op=mybir.AluOpType.mult)
                for b in range(b0, b1):
                    # per-bank partial reduction into tmp [P, NBANK*D]
                    tmp = opool.tile([P, NBANK * D], f32, tag="tmp")
                    for k in range(NBANK):
                        S = int(uni[b, k])
                        o0 = int(col_off[b, k] - col_off[b0, k])
                        if S == 0:
                            nc.vector.memset(tmp[:, k * D:(k + 1) * D], 0.0)
                            continue
                        nc.vector.tensor_reduce(
                            out=tmp[:, k * D:(k + 1) * D],
                            in_=grids[k][:, o0 * D:(o0 + S) * D].rearrange(
                                "p (s d) -> p d s", s=S),
                            axis=mybir.AxisListType.X, op=mybir.AluOpType.add)
                    agg = opool.tile([P, D], f32, tag="agg")
                    nc.vector.tensor_reduce(
                        out=agg[:],
                        in_=tmp[:].rearrange("p (k d) -> p d k", k=NBANK),
                        axis=mybir.AxisListType.X, op=mybir.AluOpType.add)
                    # out_block = agg @ M + 1s*bS  (via aggT)
                    t_p = ppool.tile([D, P], f32, tag="pt")
                    nc.tensor.transpose(t_p[:], agg[:], id_s[:, :])
                    aggT = opool.tile([D, P], f32, tag="aggT")
                    nc.vector.tensor_copy(out=aggT[:], in_=t_p[:])
                    o_p = ppool.tile([P, D], f32, tag="po")
                    nc.tensor.matmul(o_p[:], aggT[:], m_s[:], start=True, stop=False)
                    nc.tensor.matmul(o_p[:], ones_s[:, :], bs_s[:, :], start=False,
                                     stop=True, skip_group_check=True)
                    o_s = opool.tile([P, D], f32, tag="os")
                    nc.vector.tensor_copy(out=o_s[:], in_=o_p[:])
                    nc.sync.dma_start(out=out[b * P:(b + 1) * P, :], in_=o_s[:])
    nc.compile()
    return nc


def _greedy_pack(dd, nb):
    """Greedy 4-dim vector packing of len(dd) items into nb bins of <=128."""
    n = dd.shape[0]
    order = np.argsort(
        -(dd.max(1).astype(np.int64) * 1000 + dd.sum(1)), kind="stable")
    caps = np.zeros((nb, NBANK), np.int32)
    cnt = np.zeros(nb, np.int32)
    capsum = np.zeros(nb, np.int64)
    assign = np.empty(n, np.int32)
    big = 1 << 30
    for i in order:
        v = dd[i]
        inc = np.maximum(caps, v).sum(1) - capsum
        inc[cnt >= P] = big
        j = int(np.argmin(inc))
        caps[j] = np.maximum(caps[j], v)
        capsum[j] = caps[j].sum()
        cnt[j] += 1
        assign[i] = j
    return caps, assign


def kernel(x, edge_index, W, b, causal_attention, L=1, **_unused):
    global LAST_EXEC_NS
    x = np.ascontiguousarray(np.asarray(x, dtype=np.float32))
    ei = np.asarray(edge_index, dtype=np.int64)
    W = np.asarray(W, dtype=np.float32)
    bb = np.asarray(b, dtype=np.float32).reshape(D, 1)
    ca = np.asarray(causal_attention, dtype=np.float32)
    N = x.shape[0]
    src, dst = ei[0], ei[1]

    # GCN normalization (index-only math)
    deg = np.bincount(dst, minlength=N).astype(np.float64) + 1.0
    dinv = (1.0 / np.sqrt(deg)).astype(np.float32)

    # per-dst per-bank in-degree (self loop counts in bank(dst))
    bank_e = (src // BANK_ROWS).astype(np.int32)
    degkb = np.zeros((N, NBANK), np.int32)
    for k in range(NBANK):
        np.add.at(degkb[:, k], dst[bank_e == k], 1)
    degkb[np.arange(N), np.arange(N) // BANK_ROWS] += 1

    n_per = N // N_CORES
    n_blocks = (n_per + P - 1) // P
    total_blocks = n_blocks * N_CORES

    # greedy pack in 8 slices (speed), then snake-deal blocks to cores
    all_caps = np.zeros((total_blocks, NBANK), np.int32)
    assign = np.empty(N, np.int64)  # dst -> global block id
    for g in range(N_CORES):
        sl = slice(g * n_per, (g + 1) * n_per)
        caps, asg = _greedy_pack(degkb[sl], n_blocks)
        all_caps[g * n_blocks:(g + 1) * n_blocks] = caps
        assign[sl] = asg + g * n_blocks
    o = np.argsort(-all_caps.sum(1), kind="stable")
    # dealt[c, r] = global block id at position r on core c (snake)
    dealt = np.empty((N_CORES, n_blocks), np.int64)
    for r in range(n_blocks):
        blocks = o[r * N_CORES:(r + 1) * N_CORES]
        if r % 2 == 1:
            blocks = blocks[::-1]
        dealt[:, r] = blocks
    uni = all_caps[dealt].max(axis=0)  # [n_blocks, NBANK]
    col_off = np.zeros((n_blocks + 1, NBANK), np.int64)
    col_off[1:] = np.cumsum(uni, axis=0)
    ST = col_off[-1]

    # map: global block id -> (core, position)
    blk_core = np.empty(total_blocks, np.int64)
    blk_pos = np.empty(total_blocks, np.int64)
    for c in range(N_CORES):
        blk_core[dealt[c]] = c
        blk_pos[dealt[c]] = np.arange(n_blocks)

    # lane assignment: dsts of each global block get lanes 0..cnt-1
    # order dsts by (core, pos, dst id)
    dst_block = assign  # global block per dst
    keys = blk_core[dst_block] * (n_blocks * P * 8) + blk_pos[dst_block] * P * 8
    order_d = np.argsort(keys, kind="stable")  # dsts grouped by (core, pos)
    # lane = index within group
    gb_sorted = dst_block[order_d]
    grp_start_idx = np.searchsorted(
        blk_core[gb_sorted] * n_blocks + blk_pos[gb_sorted],
        np.arange(total_blocks), side="left")
    lane_sorted = np.arange(N) - grp_start_idx[
        blk_core[gb_sorted] * n_blocks + blk_pos[gb_sorted]]
    lane = np.empty(N, np.int64)
    lane[order_d] = lane_sorted
    core_of = blk_core[dst_block]
    pos_of = blk_pos[dst_block]

    # per-edge norm values; self loops appended as edges
    norm_e = dinv[src] * dinv[dst]
    loops = np.arange(N)
    src_all = np.concatenate([loops, src])   # self loops FIRST (slot 0)
    dst_all = np.concatenate([loops, dst])
    w_all = np.concatenate([(dinv * dinv).astype(np.float32),
                            norm_e.astype(np.float32)])
    bank_all = (src_all // BANK_ROWS).astype(np.int32)

    # chunk blocks by total column budget (SBUF: grid tiles are
    # budget*256B per partition per buffer, double buffered)
    COL_BUDGET = 160
    chunks = []
    b0 = 0
    while b0 < n_blocks:
        b1 = b0 + 1
        while b1 < n_blocks and uni[b0:b1 + 1].sum() <= COL_BUDGET:
            b1 += 1
        chunks.append((b0, b1))
        b0 = b1

    nc = _build_nc(N, uni, chunks)

    in_maps = []
    perms = []  # per core: row index in out -> global dst id
    ecore = core_of[dst_all]
    epos = pos_of[dst_all]
    elane = lane[dst_all]
    for c in range(N_CORES):
        im = {"x": x, "wmat": W, "bvec": bb, "cattn": ca,
              "ident": np.eye(P, dtype=np.float32)}
        esel = ecore == c
        for k in range(NBANK):
            sel = esel & (bank_all == k)
            s_k = src_all[sel] - k * BANK_ROWS
            w_k = w_all[sel]
            pos_k = epos[sel]
            lane_k = elane[sel]
            # sort by (pos, lane) stable to get slot j within (pos, lane)
            rank = pos_k * P + lane_k
            o2 = np.argsort(rank, kind="stable")
            rank_s, s_s, w_s_, pos_s, lane_s = (
                rank[o2], s_k[o2], w_k[o2], pos_k[o2], lane_k[o2])
            grp = np.searchsorted(rank_s, np.arange(n_blocks * P), side="left")
            j_in = np.arange(len(rank_s)) - grp[rank_s]
            cols = col_off[pos_s, k] + j_in
            st_k = int(ST[k])
            offs_lin = np.zeros(st_k * P, np.int16)
            norms_arr = np.zeros((P, st_k), np.float32)
            offs_lin[cols * P + lane_s] = s_s.astype(np.int16)
            norms_arr[lane_s, cols] = w_s_
            # wrap idx: linear i -> [i%16 (+16r), i//16]
            wrapped = offs_lin.reshape(st_k * P // 16, 16).T  # [16, numk/16]
            idx_arr = np.tile(wrapped, (8, 1))
            im[f"idx{k}"] = np.ascontiguousarray(idx_arr)
            im[f"norms{k}"] = norms_arr
        in_maps.append(im)
        # perm: (pos, lane) -> dst global id
        mine = core_of == c
        pm = np.full(n_blocks * P, -1, np.int64)
        pm[pos_of[mine] * P + lane[mine]] = np.nonzero(mine)[0]
        perms.append(pm)

    trace = bool(os.environ.get("KERNEL_TRACE"))
    if trace:
        try:
            import ntff_shim  # noqa: F401
        except Exception:
            trace = False
    r = run_bass_kernel_spmd(nc, in_maps, list(range(N_CORES)), trace=trace)
    LAST_EXEC_NS = r.exec_time_ns

    out = np.empty((N, D), dtype=np.float32)
    for c in range(N_CORES):
        pm = perms[c]
        valid = pm >= 0
        out[pm[valid]] = r.results[c]["out"][valid]
    return out
